# revision 5
# baseline (speedup 1.0000x reference)
"""KAN GLU expert (nn_KAN_GLUExpert) — TRN2 Bass kernel, 8 NeuronCores.

Math: reference kan_linear(x, bw, sw, grid) = silu(x) @ bw.T + einsum('nic,oic->no', b_splines(x), sw)
with a uniform shared grid (h=0.4 on [-2.2, 2.2], cubic, 8 basis fns). On a uniform grid the
8 spline bases are shifted copies of the cardinal cubic B-spline:
    B_c(x) = r^3/6 - (2/3) q^3,  r = relu(2 - t), q = relu(1 - t),  t = |2.5 x + 3.5 - c|
(verified to fp32 rounding against the Cox-de Boor recursion, incl. exact knots / out-of-domain).

base + spline fuse into ONE matmul over K = 9*in: slab 0 = silu(x) (f32r x f32r base
weights), slabs 1..8 = B_c(x) (bf16 x bf16 spline weights; errors scaled by the 0.1 spline
weight scale). Mixed-dtype matmuls accumulate into the same PSUM bank (verified on HW).

Sharding: data-parallel over tokens. Each of the 8 cores takes 512 of the 4096 tokens and
streams all weights once; no collective. Output slices are concatenated on host.
"""
import numpy as np
import ml_dtypes

import concourse.bacc as bacc
import concourse.mybir as mybir
import concourse.tile as tile
from concourse.bass_utils import run_bass_kernel_spmd

F32 = mybir.dt.float32
F32R = mybir.dt.float32r
BF16 = mybir.dt.bfloat16
AF = mybir.ActivationFunctionType
ALU = mybir.AluOpType

DM = 1024          # d_model
DF = 4096          # d_ff
C = 8              # spline coefficients per input
NCORES = 8
TOK = 512          # tokens per core
NPAIR = 16         # L12 row-pairs: 256 rows of w1 + 256 rows of w2 each
KI1 = DM // 128    # 8 k-tiles per slab, layer 1
KI3 = DF // 128    # 32 k-tiles per slab, layer 3

SQ_A = float(6.0 ** -0.5)          # square scale so a2 = a^2/6
SQ_B = float((2.0 / 3.0) ** 0.5)   # square scale so b2 = (2/3) b^2

_BF16 = ml_dtypes.bfloat16


def _register_const(nc, value, dtype=F32):
    key = (dtype, float(value))
    if key in nc.const_aps.aps:
        return
    t = nc.alloc_sbuf_tensor(f"const-{dtype.name}-{value}", [128, 1], dtype)
    nc.gpsimd.memset(t.ap(), float(value))
    nc.const_aps.aps[key] = t.ap()


def _basis_ops(nc, ws, out_ap, x_ap, c, shape):
    """Emit ops computing B_c slab for x_ap (fp32) into out_ap (bf16).
    fp32 intermediates. Engine split: scalar Abs + 2 Squares; vector TS + 2 STT;
    gpsimd TS + final add — balances the three elementwise engines under TensorE."""
    t = ws.tile(shape, F32, tag="ws", name=f"t_{c}")
    nc.scalar.activation(t[:], x_ap, AF.Abs, bias=float(3.5 - c), scale=2.5)
    a = ws.tile(shape, F32, tag="ws", name=f"a_{c}")
    nc.vector.tensor_scalar(a[:], t[:], 2.0, 0.0, ALU.subtract, ALU.min)   # a = -r
    b = ws.tile(shape, F32, tag="ws", name=f"b_{c}")
    nc.gpsimd.tensor_scalar(b[:], t[:], 1.0, 0.0, ALU.subtract, ALU.min)   # b = -q
    a2 = ws.tile(shape, F32, tag="ws", name=f"a2_{c}")
    nc.scalar.activation(a2[:], a[:], AF.Square, scale=SQ_A)               # a^2/6
    u = ws.tile(shape, F32, tag="ws", name=f"u_{c}")
    nc.vector.scalar_tensor_tensor(u[:], a2[:], -1.0, a[:], ALU.mult, ALU.mult)  # -a^3/6 = r^3/6  (a=-r)
    b2 = ws.tile(shape, F32, tag="ws", name=f"b2_{c}")
    nc.scalar.activation(b2[:], b[:], AF.Square, scale=SQ_B)               # (2/3) b^2
    v = ws.tile(shape, F32, tag="ws", name=f"v_{c}")
    nc.vector.scalar_tensor_tensor(v[:], b2[:], 1.0, b[:], ALU.mult, ALU.mult)   # (2/3) b^3 = -(2/3) q^3
    nc.gpsimd.tensor_add(out_ap, u[:], v[:])                               # B = r^3/6 - (2/3) q^3


def build_program():
    nc = bacc.Bacc("TRN2", target_bir_lowering=False, debug=False, num_devices=NCORES)

    xs_d = nc.dram_tensor("xs", (128, KI1, TOK), F32, kind="ExternalInput")
    wb12_d = nc.dram_tensor("wb12", (NPAIR, 128, KI1, 512), F32R, kind="ExternalInput")
    ws12_d = nc.dram_tensor("ws12", (NPAIR, C, 128, KI1, 512), BF16, kind="ExternalInput")
    wb3_d = nc.dram_tensor("wb3", (16, 128, 2, 1024), F32R, kind="ExternalInput")
    ws3_d = nc.dram_tensor("ws3", (C, 8, 128, 4, 1024), BF16, kind="ExternalInput")
    out_d = nc.dram_tensor("out", (128, 8, TOK), F32, kind="ExternalOutput")

    for c in range(C):
        _register_const(nc, 3.5 - c)
    nc.all_engine_barrier()

    with tile.TileContext(nc) as tc:
        with tc.tile_pool(name="hpool", bufs=1) as hpool:
            h = hpool.tile([128, KI3, TOK], F32, name="h")

            # ---------------- layers 1+2 (GLU halves) ----------------
            with (
                tc.tile_pool(name="slabs1", bufs=1) as slabs1,
                tc.tile_pool(name="wload_b", bufs=2) as wload_b,
                tc.tile_pool(name="wload_s", bufs=2) as wload_s,
                tc.tile_pool(name="ps12", bufs=8, space="PSUM") as ps12,
                tc.tile_pool(name="glu_tmp", bufs=2) as glu_tmp,
            ):
                silu1 = slabs1.tile([128, KI1, TOK], F32R, name="silu1")
                B1 = [slabs1.tile([128, KI1, TOK], BF16, tag=f"B1_{c}", name=f"B1_{c}")
                      for c in range(C)]

                # basis over x, strips of 1 k-tile (SBUF-tight phase: h+slabs resident)
                with (
                    tc.tile_pool(name="xload", bufs=1) as xload,
                    tc.tile_pool(name="ws1", bufs=6) as ws1,
                ):
                    for s in range(KI1):
                        xt = xload.tile([128, 1, TOK], F32, tag="x", name=f"x_{s}")
                        nc.sync.dma_start(xt[:], xs_d[:, s:s + 1, :])
                        nc.scalar.activation(silu1[:, s:s + 1, :], xt[:], AF.Silu)
                        for c in range(C):
                            _basis_ops(nc, ws1, B1[c][:, s:s + 1, :], xt[:],
                                       c, [128, 1, TOK])

                for j in range(NPAIR):
                    acc = [ps12.tile([128, TOK], F32, tag="ps", name=f"ps_{j}_{m}")
                           for m in range(4)]
                    # base part: f32r silu slab x f32r base weights
                    for q in range(KI1 // 2):
                        wbq = wload_b.tile([128, 2, 512], F32R, tag="wb", name=f"wb_{j}_{q}")
                        nc.sync.dma_start(wbq[:], wb12_d[j, :, 2 * q:2 * q + 2, :])
                        for r in range(2):
                            ki = 2 * q + r
                            for m in range(4):
                                nc.tensor.matmul(
                                    acc[m][:], wbq[:, r, 128 * m:128 * (m + 1)],
                                    silu1[:, ki, :], start=(ki == 0), stop=False)
                    # spline part: bf16 B slabs x bf16 spline weights
                    for c in range(C):
                        wsb = wload_s.tile([128, KI1, 512], BF16, tag="ws", name=f"wsl_{j}_{c}")
                        nc.sync.dma_start(wsb[:], ws12_d[j, c])
                        for ki in range(KI1):
                            last = (c == C - 1 and ki == KI1 - 1)
                            for m in range(4):
                                nc.tensor.matmul(
                                    acc[m][:], wsb[:, ki, 128 * m:128 * (m + 1)],
                                    B1[c][:, ki, :], start=False, stop=last)
                    # GLU: h rows [256j, 256j+256) = L1 * L2
                    for t in range(2):
                        tmp = glu_tmp.tile([128, TOK], F32, tag="gt", name=f"gt_{j}_{t}")
                        nc.scalar.copy(tmp[:], acc[t][:])
                        nc.vector.tensor_mul(h[:, 2 * j + t, :], tmp[:], acc[2 + t][:])

            # ---------------- layer 3 ----------------
            with (
                tc.tile_pool(name="sil3", bufs=2) as sil3p,
                tc.tile_pool(name="b3", bufs=3) as b3p,
                tc.tile_pool(name="ws3", bufs=6) as ws3,
                tc.tile_pool(name="w3load_b", bufs=2) as w3load_b,
                tc.tile_pool(name="w3load_s", bufs=2) as w3load_s,
                tc.tile_pool(name="ps3", bufs=1, space="PSUM") as ps3,
                tc.tile_pool(name="outp", bufs=1) as outp,
            ):
                acc3 = [ps3.tile([128, TOK], F32, tag=f"o{m}", name=f"ps3_{m}")
                        for m in range(8)]
                # base part: silu(h) strips of 4 k-tiles
                for s in range(KI3 // 4):
                    sil = sil3p.tile([128, 4, TOK], F32R, tag="sil", name=f"sil_{s}")
                    nc.scalar.activation(sil[:], h[:, 4 * s:4 * s + 4, :], AF.Silu)
                    for half in range(2):
                        wt = w3load_b.tile([128, 2, 1024], F32R, tag="w3b", name=f"w3b_{s}_{half}")
                        nc.sync.dma_start(wt[:], wb3_d[2 * s + half])
                        for r in range(2):
                            ki = 4 * s + 2 * half + r
                            for m in range(8):
                                nc.tensor.matmul(
                                    acc3[m][:], wt[:, r, 128 * m:128 * (m + 1)],
                                    sil[:, 2 * half + r, :], start=(ki == 0), stop=False)
                # spline part (basis in half-strips of 2 k-tiles to bound workspace)
                for c in range(C):
                    for s in range(KI3 // 4):
                        bt = b3p.tile([128, 4, TOK], BF16, tag="b3", name=f"b3_{c}_{s}")
                        for half in range(2):
                            _basis_ops(nc, ws3, bt[:, 2 * half:2 * half + 2, :],
                                       h[:, 4 * s + 2 * half:4 * s + 2 * half + 2, :],
                                       c, [128, 2, TOK])
                        wt = w3load_s.tile([128, 4, 1024], BF16, tag="w3s", name=f"w3s_{c}_{s}")
                        nc.sync.dma_start(wt[:], ws3_d[c, s])
                        last_cs = (c == C - 1 and s == KI3 // 4 - 1)
                        for r in range(4):
                            for m in range(8):
                                nc.tensor.matmul(
                                    acc3[m][:], wt[:, r, 128 * m:128 * (m + 1)],
                                    bt[:, r, :], start=False,
                                    stop=(last_cs and r == 3))
                # copy out
                ostage = outp.tile([128, 8, TOK], F32, name="ostage")
                for m in range(8):
                    nc.scalar.copy(ostage[:, m, :], acc3[m][:])
                nc.sync.dma_start(out_d[:], ostage[:])

    nc.compile()
    return nc


def pack_weights(base_w1, spline_w1, base_w2, spline_w2, base_w3, spline_w3):
    f32 = np.float32
    # WB12: (16, 128, 8, 512) — cols = [w1 rows 256j..  , w2 rows 256j..]
    w12 = np.concatenate([np.asarray(base_w1, f32).reshape(NPAIR, 256, DM),
                          np.asarray(base_w2, f32).reshape(NPAIR, 256, DM)], axis=1)  # (16, 512, 1024) [j, m, k]
    wb12 = np.ascontiguousarray(
        w12.reshape(NPAIR, 512, KI1, 128).transpose(0, 3, 2, 1))  # (16, 128, 8, 512)

    # WS12: (16, 8, 128, 8, 512) bf16
    s12 = np.concatenate([np.asarray(spline_w1, f32).reshape(NPAIR, 256, DM, C),
                          np.asarray(spline_w2, f32).reshape(NPAIR, 256, DM, C)], axis=1)  # (16, 512, 1024, 8)
    ws12 = np.ascontiguousarray(
        s12.reshape(NPAIR, 512, KI1, 128, C).transpose(0, 4, 3, 2, 1)).astype(_BF16)  # (16, C, 128, 8, 512)

    # WB3: (16, 128, 2, 1024): base_w3 (1024, 4096): [m, k]
    wb3 = np.ascontiguousarray(
        np.asarray(base_w3, f32).T.reshape(16, 2, 128, DM).transpose(0, 2, 1, 3))  # (16, 128, 2, 1024)

    # WS3: (8, 8, 128, 4, 1024) bf16: spline_w3 (1024, 4096, 8)
    ws3 = np.ascontiguousarray(
        np.asarray(spline_w3, f32).transpose(2, 1, 0)    # (8, 4096, 1024)
        .reshape(C, 8, 4, 128, DM).transpose(0, 1, 3, 2, 4)).astype(_BF16)  # (8, 8, 128, 4, 1024)
    return wb12, ws12, wb3, ws3


_prog_cache = {}


def kernel(x, base_w1, spline_w1, base_w2, spline_w2, base_w3, spline_w3,
           grid_in=None, grid_ff=None):
    x = np.asarray(x, np.float32)
    shp = x.shape
    x2 = x.reshape(-1, DM)                       # (4096, 1024)
    ntok = x2.shape[0]
    assert ntok == NCORES * TOK

    wb12, ws12, wb3, ws3 = pack_weights(base_w1, spline_w1, base_w2,
                                        spline_w2, base_w3, spline_w3)

    if "nc" not in _prog_cache:
        _prog_cache["nc"] = build_program()
    nc = _prog_cache["nc"]

    in_maps = []
    for cidx in range(NCORES):
        xs = np.ascontiguousarray(
            x2[cidx * TOK:(cidx + 1) * TOK].T.reshape(KI1, 128, TOK).transpose(1, 0, 2))
        in_maps.append({"xs": xs, "wb12": wb12, "ws12": ws12, "wb3": wb3, "ws3": ws3})

    res = run_bass_kernel_spmd(nc, in_maps, core_ids=list(range(NCORES)))

    out = np.empty((ntok, DM), np.float32)
    for cidx in range(NCORES):
        o = res.results[cidx]["out"]             # (128, 8, 512)
        out[cidx * TOK:(cidx + 1) * TOK] = o.transpose(1, 0, 2).reshape(DM, TOK).T
    return out.reshape(shp)
